# revision 2
# baseline (speedup 1.0000x reference)
"""GCN diag-encoder (2-layer SpMM) on 8 Trainium2 NeuronCores.

Strategy: the sparse adjacency (640K edges over 10K nodes, ~0.64% dense) is
materialized as a dense A^T matrix on the host and the per-layer
  out[dst] = sum_e vals[e] * x[src[e]]        (segment-sum SpMM)
becomes a dense matmul  out1_t = x^T-tiles contracted with A^T-tiles on the
TensorEngine:
  matmul(out=psum[feat, dst], lhsT=x_tile[src,feat], rhs=AT_tile[src,dst])
Each core owns a 1250-column (dst) slice of A^T (padded to 1280) and
accumulates its whole [128 feat x 1280 dst] output in PSUM while streaming
A^T k-tiles from HBM in bf16.  Between the two layers: tanh on the scalar
engine, diag-W scale, PE-transpose to node-major, AllGather across the 8
cores, then layer 2 with a row-permuted copy of A^T matched to the gathered
layout.  W (diag weights) are folded: W0 into x on the host, W1 applied at
the layer-1 eviction.
"""

import numpy as np
import ml_dtypes

N = 10000          # nodes
D = 128            # feature dim
NCORES = 8
S = 1250           # dst nodes per core
SP = 1280          # padded dst per core (10 tiles of 128)
KT = 80            # contraction k-tiles (padded src rows = 10240)
NPAD = KT * 128    # 10240
G = 10             # k-tile groups per layer (DMA batches)
KPG = 8            # k-tiles per group
BF16 = ml_dtypes.bfloat16

_PROG_CACHE = {}


def _build_program():
    import concourse.bacc as bacc
    import concourse.mybir as mybir
    from concourse import tile

    f32 = mybir.dt.float32
    bf16 = mybir.dt.bfloat16

    nc = bacc.Bacc(
        "TRN2",
        target_bir_lowering=False,
        debug=False,
        enable_asserts=False,
        num_devices=NCORES,
    )

    a1 = nc.dram_tensor("a1", [G, 128, KPG * SP], bf16, kind="ExternalInput").ap()
    a2 = nc.dram_tensor("a2", [G, 128, KPG * SP], bf16, kind="ExternalInput").ap()
    x0 = nc.dram_tensor("x0", [128, NPAD], bf16, kind="ExternalInput").ap()
    w1 = nc.dram_tensor("w1", [128, 1], f32, kind="ExternalInput").ap()
    ident = nc.dram_tensor("ident", [128, 128], f32, kind="ExternalInput").ap()
    out = nc.dram_tensor("out", [128, SP], f32, kind="ExternalOutput").ap()

    with tile.TileContext(nc) as tc:
        with (
            tc.tile_pool(name="xp", bufs=1) as xp,
            tc.tile_pool(name="ab", bufs=3) as apool,
            tc.tile_pool(name="ev", bufs=1) as ev,
            tc.tile_pool(name="ps", bufs=1, space="PSUM") as ps,
            tc.tile_pool(name="pt", bufs=2, space="PSUM") as pt,
            tc.tile_pool(name="dr", bufs=1, space="DRAM") as dr,
        ):
            x0s = xp.tile([128, NPAD], bf16, tag="x0s")
            x1s = xp.tile([128, NPAD], bf16, tag="x1s")
            w1c = xp.tile([128, 1], f32, tag="w1c")
            idn = xp.tile([128, 128], f32, tag="idn")
            nc.scalar.dma_start(x0s[:], x0)
            nc.scalar.dma_start(w1c[:], w1)
            nc.scalar.dma_start(idn[:], ident)

            agin = dr.tile([128, SP], bf16)
            agout = dr.tile([NCORES * 128, SP], bf16, addr_space="Shared")

            def do_layer(a_dram, xs, psum):
                for g in range(G):
                    ab = apool.tile([128, KPG * SP], bf16, tag="ab")
                    nc.sync.dma_start(ab[:], a_dram[g])
                    for kk in range(KPG):
                        k = g * KPG + kk
                        lhsT = xs[:, k * 128:(k + 1) * 128]
                        for c0, cn in ((0, 512), (512, 512), (1024, 256)):
                            nc.tensor.matmul(
                                psum[:, c0:c0 + cn],
                                lhsT,
                                ab[:, kk * SP + c0: kk * SP + c0 + cn],
                                start=(k == 0),
                                stop=(k == KT - 1),
                            )

            # ---- layer 1 ----
            psum1 = ps.tile([128, SP], f32, tag="acc1")
            do_layer(a1, x0s, psum1)

            # evict: x1 = tanh(psum1) * W1  (feat on partitions -> W1 is a
            # per-partition scalar), then transpose each 128x128 tile to
            # node-major for the AllGather.
            x1f = ev.tile([128, SP], f32, tag="x1f")
            nc.scalar.activation(
                x1f[:], psum1[:], mybir.ActivationFunctionType.Tanh
            )
            nc.vector.tensor_scalar_mul(x1f[:], x1f[:], w1c[:])
            agin_sb = ev.tile([128, SP], bf16, tag="agin")
            for t in range(10):
                tp = pt.tile([128, 128], f32, tag="tp")
                nc.tensor.transpose(tp[:], x1f[:, t * 128:(t + 1) * 128], idn[:])
                nc.vector.tensor_copy(agin_sb[:, t * 128:(t + 1) * 128], tp[:])
            nc.scalar.dma_start(agin[:], agin_sb[:])

            nc.gpsimd.collective_compute(
                "AllGather",
                mybir.AluOpType.bypass,
                replica_groups=[list(range(NCORES))],
                ins=[agin.opt()],
                outs=[agout.opt()],
            )
            # agout rows r*128..(r+1)*128 = rank r's node-major shard; lay
            # them side-by-side in the free dim to form layer-2 lhsT tiles.
            nc.sync.dma_start(
                x1s[:].rearrange("p (r j) -> p r j", r=NCORES),
                agout[:].rearrange("(r p) j -> p r j", p=128),
            )

            # ---- layer 2 ----
            psum2 = ps.tile([128, SP], f32, tag="acc2")
            do_layer(a2, x1s, psum2)
            ob = ev.tile([128, SP], f32, tag="ob")
            nc.vector.tensor_copy(ob[:], psum2[:])
            nc.sync.dma_start(out, ob[:])

    nc.compile()
    return nc


def get_program():
    if "nc" not in _PROG_CACHE:
        _PROG_CACHE["nc"] = _build_program()
    return _PROG_CACHE["nc"]


def build_in_maps(x, src, dst, vals, W):
    """Host-side prep: dense A^T shards (bf16) + arranged x0."""
    import scipy.sparse as sp

    x = np.asarray(x, np.float32)
    src = np.asarray(src, np.int64)
    dst = np.asarray(dst, np.int64)
    vals = np.asarray(vals, np.float32)
    W = np.asarray(W, np.float32)

    # A[dst, src] = sum of vals  ->  we build AT[src, dst]
    AT = sp.coo_matrix((vals, (src, dst)), shape=(N, N)).toarray()

    xw = x * W[0][None, :]
    x0h = np.zeros((NPAD, D), np.float32)
    x0h[:N] = xw
    # [p, (k f)] layout: col-block k holds feats of node k*128+p
    x0h = np.ascontiguousarray(
        x0h.reshape(KT, 128, D).transpose(1, 0, 2).reshape(128, KT * D)
    ).astype(BF16)

    # layer-2 contraction-row permutation: row i <- global node of
    # (rank r = i//1280, tile t = (i%1280)//128, p = i%128)
    i2 = np.arange(NPAD)
    r2 = i2 // SP
    loc = i2 % SP
    node2 = r2 * S + loc
    valid2 = loc < S
    node2c = np.where(valid2, node2, 0)

    w1col = np.ascontiguousarray(W[1].reshape(128, 1)).astype(np.float32)
    ident = np.eye(128, dtype=np.float32)

    def arrange(a_pad16):
        # [NPAD, SP] -> [G, 128, KPG*SP] with [g, p, kk*SP+j] = row g*1024+kk*128+p
        return np.ascontiguousarray(
            a_pad16.reshape(G, KPG, 128, SP).transpose(0, 2, 1, 3).reshape(
                G, 128, KPG * SP
            )
        )

    in_maps = []
    for c in range(NCORES):
        ATc = np.zeros((NPAD, SP), np.float32)
        ATc[:N, :S] = AT[:, c * S:(c + 1) * S]
        ATc16 = ATc.astype(BF16)
        AT2 = ATc16[node2c]
        AT2[~valid2] = 0
        in_maps.append(
            {
                "a1": arrange(ATc16),
                "a2": arrange(AT2),
                "x0": x0h,
                "w1": w1col,
                "ident": ident,
            }
        )
    return in_maps


def assemble_output(results):
    outs = []
    for c in range(NCORES):
        ot = np.asarray(results[c]["out"], np.float32)  # [128, SP] feat-major
        outs.append(ot[:, :S].T)
    return np.ascontiguousarray(np.concatenate(outs, axis=0))


def kernel(x, src, dst, vals, W):
    from concourse import bass_utils

    nc = get_program()
    in_maps = build_in_maps(x, src, dst, vals, W)
    res = bass_utils.run_bass_kernel_spmd(
        nc, in_maps, core_ids=list(range(NCORES))
    )
    return assemble_output(res.results)
